# revision 21
# baseline (speedup 1.0000x reference)
"""PerceiverAttentionCA on 8 NeuronCores (Trainium2, Bass/Tile).

Sharding: core i handles batch b = i // 4 and head-group g = i % 4
(4 of the 16 heads = 512 of the 2048 inner columns).  Each core
computes LN1(x[b]) / LN2(latents[b]) in fp32, projects q/k/v for its
heads with bf16 matmuls (LayerNorm gamma and the attention scale are
folded into the weights on the host, beta folded into per-projection
bias vectors), runs softmax attention, and produces the partial output
O_g @ w_out[g-rows].  The host sums the 4 partials per batch.

Structure (v2): LN and projections are interleaved per n2/n1 quarter so
the tensor engine is fed from ~30us onward (HAM stays ramped);
activations cross the PE K-major via HWDGE DMA-transpose (no PE
transpose cycles); softmax skips max-subtraction (scores are O(10)
here; exp stays far inside fp32 range) and gets its denominator from
the activation accumulator; attention and the final projection are
interleaved per 512-row block to keep PE dense through the tail.
"""

import numpy as np
import ml_dtypes

from contextlib import ExitStack

import concourse.bass as bass
import concourse.tile as tile
from concourse import bacc, mybir
from concourse.bass_utils import run_bass_kernel_spmd
from concourse.masks import make_identity

FP32 = mybir.dt.float32
BF16 = mybir.dt.bfloat16
AF = mybir.ActivationFunctionType
ALU = mybir.AluOpType

P = 128
B = 2
N1 = 2048          # x sequence length
N2 = 2048          # latents sequence length
KV = 2048          # x feature dim
D = 3072           # latents feature dim
HEADS = 16
DH = 128
INNER = HEADS * DH  # 2048
G = 4               # head groups (tensor-parallel degree per batch)
HC = HEADS // G     # heads per core = 4
IC = HC * DH        # inner columns per core = 512
EPS = 1e-5

N1C = N1 // P       # 16 n1 chunks of 128
N2C = N2 // P       # 16 n2 chunks of 128
DC = D // P         # 24 K-chunks for q projection
KVC = KV // P       # 16 K-chunks for kv projection
OC = D // 512       # 6 out-column chunks of 512
NQ = 4              # quarters

bf16 = ml_dtypes.bfloat16


def _emit(nc, tc, io, ctx):
    x_d = io["x_b"]
    lat_d = io["lat_b"]
    wq_d = io["w_q"]
    wk_d = io["w_k"]
    wv_d = io["w_v"]
    wo_d = io["w_o"]
    bq_d = io["b_q"]
    bk_d = io["b_k"]
    bv_d = io["b_v"]
    out_d = io["out_b"]

    const = ctx.enter_context(tc.tile_pool(name="const", bufs=1))
    stat = ctx.enter_context(tc.tile_pool(name="stat", bufs=8))

    # persistent activations (bf16, K-major unless noted)
    persist = ctx.enter_context(tc.tile_pool(name="persist", bufs=1))
    qT = persist.tile([P, G, N2], BF16, tag="qT")     # [dh, head, n2]
    kT = persist.tile([P, G, N1], BF16, tag="kT")     # [dh, head, n1]
    vn = persist.tile([P, N1C, IC], BF16, tag="vn")   # natural [n1, inner]

    eps_t = const.tile([P, 1], FP32)
    nc.vector.memset(eps_t, EPS)
    ones_t = const.tile([P, P], BF16)
    nc.vector.memset(ones_t, 1.0)
    idn = const.tile([P, P], BF16)
    make_identity(nc, idn)
    bq_t = const.tile([P, G], FP32)
    bk_t = const.tile([P, G], FP32)
    bv_rep = const.tile([P, IC], FP32)

    def ln_tile(xt, dim):
        """LayerNorm stats for one [128, dim] fp32 tile -> (mu, rstd)."""
        nsub = dim // 512
        st = stat.tile([P, nsub, 6], FP32, tag="st")
        for s in range(nsub):
            nc.vector.bn_stats(out=st[:, s, :], in_=xt[:, s * 512:(s + 1) * 512])
        mv = stat.tile([P, 2], FP32, tag="mv")
        nc.vector.bn_aggr(out=mv, in_=st)
        rstd = stat.tile([P, 1], FP32, tag="rstd")
        nc.scalar.activation(out=rstd, in_=mv[:, 1:2], func=AF.Sqrt, bias=eps_t)
        nc.vector.reciprocal(out=rstd, in_=rstd)
        nmr = stat.tile([P, 1], FP32, tag="nmr")
        nc.vector.tensor_mul(out=nmr, in0=mv[:, 0:1], in1=rstd)
        nc.vector.tensor_scalar_mul(out=nmr, in0=nmr, scalar1=-1.0)
        return rstd, nmr

    def pe_warmup(n_mm=96):
        """Dummy matmuls to fill the LN prologue and ramp the PE clock (HAM)
        before the first real projection; results are discarded."""
        with tc.tile_pool(name="warm", bufs=1) as wpool, \
             tc.tile_pool(name="warm_psum", bufs=1, space="PSUM") as wpsum:
            wa = wpool.tile([P, 512], BF16, tag="warm")
            nc.vector.memset(wa, 0.25)
            wp = wpsum.tile([P, 512], FP32, tag="warmp")
            for i in range(n_mm):
                nc.tensor.matmul(wp, lhsT=wa[:, :P], rhs=wa,
                                 start=(i == 0), stop=(i == n_mm - 1))

    # ---- Phase A/B: quarterized LN + q/k/v projections ----
    with tc.tile_pool(name="lat", bufs=2) as lat_pool, \
         tc.tile_pool(name="xb", bufs=2) as x_pool, \
         tc.tile_pool(name="zt", bufs=2) as z_pool, \
         tc.tile_pool(name="lnT", bufs=1) as lnT_pool, \
         tc.tile_pool(name="xnT", bufs=1) as xnT_pool, \
         tc.tile_pool(name="wq", bufs=2) as wq_pool, \
         tc.tile_pool(name="wk", bufs=2) as wk_pool, \
         tc.tile_pool(name="wv", bufs=1) as wv_pool, \
         tc.tile_pool(name="ab_psum", bufs=4, space="PSUM") as ab_psum, \
         tc.tile_pool(name="t_psum", bufs=2, space="PSUM") as t_psum:

        wq_r = wq_d.rearrange("(c p) m -> p c m", p=P)
        wk_r = wk_d.rearrange("(c p) m -> p c m", p=P)
        wv_sb = wv_pool.tile([P, KVC, IC], BF16, tag="wv")

        pe_warmup(96)

        for q4 in range(NQ):
            # LN2 quarter -> lnTq [P, DC, 512]  (z on DVE, transpose on ACT)
            lnTq = lnT_pool.tile([P, DC, 512], BF16, tag="lnT")
            for ii in range(4):
                i = q4 * 4 + ii
                xt = lat_pool.tile([P, D], FP32, tag="lat")
                nc.sync.dma_start(out=xt, in_=lat_d[i * P:(i + 1) * P, :])
                rstd, nmr = ln_tile(xt, D)
                zt = z_pool.tile([P, D], BF16, tag="zt")
                nc.scalar.activation(out=zt, in_=xt, func=AF.Identity,
                                     bias=nmr, scale=rstd)
                for cg in range(DC // 4):
                    tp = t_psum.tile([P, 4, P], BF16, tag="tp")
                    for cc in range(4):
                        c = cg * 4 + cc
                        nc.tensor.transpose(tp[:, cc, :],
                                            zt[:, c * P:(c + 1) * P], idn)
                    nc.scalar.copy(
                        out=lnTq[:, cg * 4:(cg + 1) * 4, ii * P:(ii + 1) * P],
                        in_=tp)
            # q projection for this n2 quarter
            if q4 == 0:
                nc.sync.dma_start(out=bq_t, in_=bq_d.rearrange("(c p) -> p c", p=P))
            for m in range(G):
                wqm = wq_pool.tile([P, DC, P], BF16, tag="wq")
                nc.sync.dma_start(out=wqm, in_=wq_r[:, :, m * P:(m + 1) * P])
                ps = ab_psum.tile([P, 512], FP32, tag="ab")
                for c in range(DC):
                    nc.tensor.matmul(ps, lhsT=wqm[:, c, :], rhs=lnTq[:, c, :],
                                     start=(c == 0), stop=(c == DC - 1))
                nc.scalar.activation(out=qT[:, m, q4 * 512:(q4 + 1) * 512],
                                     in_=ps, func=AF.Identity,
                                     bias=bq_t[:, m:m + 1])
            # LN1 quarter -> xnTq [P, KVC, 512]  (z on DVE, transpose on ACT)
            xnTq = xnT_pool.tile([P, KVC, 512], BF16, tag="xnT")
            for ii in range(4):
                i = q4 * 4 + ii
                xt = x_pool.tile([P, KV], FP32, tag="xb")
                nc.sync.dma_start(out=xt, in_=x_d[i * P:(i + 1) * P, :])
                rstd, nmr = ln_tile(xt, KV)
                zt = z_pool.tile([P, KV], BF16, tag="zt")
                nc.scalar.activation(out=zt, in_=xt, func=AF.Identity,
                                     bias=nmr, scale=rstd)
                nc.scalar.dma_start(out=xnTq[:, :, ii * P:(ii + 1) * P], in_=zt,
                                    transpose=True)
            # k projection for this n1 quarter
            if q4 == 0:
                nc.sync.dma_start(out=bk_t, in_=bk_d.rearrange("(c p) -> p c", p=P))
            for m in range(G):
                wkm = wk_pool.tile([P, KVC, P], BF16, tag="wk")
                nc.sync.dma_start(out=wkm, in_=wk_r[:, :, m * P:(m + 1) * P])
                ps = ab_psum.tile([P, 512], FP32, tag="ab")
                for c in range(KVC):
                    nc.tensor.matmul(ps, lhsT=wkm[:, c, :], rhs=xnTq[:, c, :],
                                     start=(c == 0), stop=(c == KVC - 1))
                nc.scalar.activation(out=kT[:, m, q4 * 512:(q4 + 1) * 512],
                                     in_=ps, func=AF.Identity,
                                     bias=bk_t[:, m:m + 1])
            # v rows for this n1 quarter (natural layout)
            if q4 == 0:
                nc.sync.dma_start(
                    out=bv_rep,
                    in_=bass.AP(tensor=bv_d.tensor, offset=bv_d.offset,
                                ap=[[0, P]] + list(bv_d.ap)))
                nc.sync.dma_start(out=wv_sb,
                                  in_=wv_d.rearrange("(c p) m -> p c m", p=P))
            for ii in range(4):
                ps = ab_psum.tile([P, IC], FP32, tag="ab")
                for c in range(KVC):
                    nc.tensor.matmul(ps, lhsT=xnTq[:, c, ii * P:(ii + 1) * P],
                                     rhs=wv_sb[:, c, :],
                                     start=(c == 0), stop=(c == KVC - 1))
                nc.vector.tensor_add(out=vn[:, q4 * 4 + ii, :], in0=ps, in1=bv_rep)

    # ---- Phase C/D: attention + output projection, per 512-row n2 block ----
    with tc.tile_pool(name="PT", bufs=4) as PT_pool, \
         tc.tile_pool(name="rep", bufs=4) as rep_pool, \
         tc.tile_pool(name="OTb", bufs=2) as OTb_pool, \
         tc.tile_pool(name="wo", bufs=1) as wo_pool, \
         tc.tile_pool(name="orow", bufs=2) as orow_pool, \
         tc.tile_pool(name="s_psum", bufs=2, space="PSUM") as s_psum, \
         tc.tile_pool(name="den_psum", bufs=1, space="PSUM") as den_psum, \
         tc.tile_pool(name="o_psum", bufs=1, space="PSUM") as o_psum, \
         tc.tile_pool(name="d_psum", bufs=2, space="PSUM") as d_psum:

        wo_sb = wo_pool.tile([P, G, D], BF16, tag="wo")
        nc.sync.dma_start(out=wo_sb, in_=wo_d.rearrange("(h p) o -> p h o", p=P))

        for nb in range(4):
            OTb = OTb_pool.tile([P, G, 512], BF16, tag="OTb")
            PTbs = []
            reps = []
            # S^T/exp for all 4 heads first: exp writes P^T directly (no
            # transpose needed).
            for h in range(G):
                PTb = PT_pool.tile([P, N1C, 512], BF16, tag="PT")
                PTbs.append(PTb)
                for j2 in range(N1C // 2):
                    ps = s_psum.tile([P, 1024], FP32, tag="s")
                    for jj in range(2):
                        j = j2 * 2 + jj
                        nc.tensor.matmul(ps[:, jj * 512:(jj + 1) * 512],
                                         lhsT=kT[:, h, j * P:(j + 1) * P],
                                         rhs=qT[:, h, nb * 512:(nb + 1) * 512],
                                         start=True, stop=True)
                    nc.scalar.activation(out=PTb[:, j2 * 2:j2 * 2 + 2, :],
                                         in_=ps, func=AF.Exp)
            # denominator: ones-matrix lhsT broadcast-reduces P^T into all
            # 128 partitions at the same PE cost; reciprocal is full-width.
            for h in range(G):
                dn = den_psum.tile([P, 512], FP32, tag="den")
                for j in range(N1C):
                    nc.tensor.matmul(dn, lhsT=ones_t, rhs=PTbs[h][:, j, :],
                                     start=(j == 0), stop=(j == N1C - 1))
                rep = rep_pool.tile([P, 512], FP32, tag="rep")
                nc.vector.reciprocal(out=rep, in_=dn)
                reps.append(rep)
            for h in range(G):
                po = o_psum.tile([P, 512], FP32, tag="o")
                for j in range(N1C):
                    nc.tensor.matmul(po, lhsT=vn[:, j, h * DH:(h + 1) * DH],
                                     rhs=PTbs[h][:, j, :],
                                     start=(j == 0), stop=(j == N1C - 1))
                nc.vector.tensor_mul(out=OTb[:, h, :], in0=po, in1=reps[h])
            # output projection for this block's 4 row-chunks
            for ii in range(4):
                orow = orow_pool.tile([P, D], FP32, tag="orow")
                for oc in range(OC):
                    ps = d_psum.tile([P, 512], FP32, tag="d")
                    for h in range(G):
                        nc.tensor.matmul(ps, lhsT=OTb[:, h, ii * P:(ii + 1) * P],
                                         rhs=wo_sb[:, h, oc * 512:(oc + 1) * 512],
                                         start=(h == 0), stop=(h == G - 1))
                    nc.vector.tensor_copy(out=orow[:, oc * 512:(oc + 1) * 512], in_=ps)
                i = nb * 4 + ii
                nc.sync.dma_start(out=out_d[i * P:(i + 1) * P, :], in_=orow)


def build_nc():
    nc = bacc.Bacc("TRN2", target_bir_lowering=False, debug=False,
                   enable_asserts=False)
    io = {
        "x_b": nc.dram_tensor("x_b", (N1, KV), FP32, kind="ExternalInput").ap(),
        "lat_b": nc.dram_tensor("lat_b", (N2, D), FP32, kind="ExternalInput").ap(),
        "w_q": nc.dram_tensor("w_q", (D, IC), BF16, kind="ExternalInput").ap(),
        "w_k": nc.dram_tensor("w_k", (KV, IC), BF16, kind="ExternalInput").ap(),
        "w_v": nc.dram_tensor("w_v", (KV, IC), BF16, kind="ExternalInput").ap(),
        "w_o": nc.dram_tensor("w_o", (IC, D), BF16, kind="ExternalInput").ap(),
        "b_q": nc.dram_tensor("b_q", (IC,), FP32, kind="ExternalInput").ap(),
        "b_k": nc.dram_tensor("b_k", (IC,), FP32, kind="ExternalInput").ap(),
        "b_v": nc.dram_tensor("b_v", (IC,), FP32, kind="ExternalInput").ap(),
        "out_b": nc.dram_tensor("out_b", (N2, D), FP32, kind="ExternalOutput").ap(),
    }
    with tile.TileContext(nc) as tc, ExitStack() as ctx:
        _emit(nc, tc, io, ctx)
    nc.compile()
    return nc


def make_in_maps(x, latents, w_q, w_kv, w_out, ln1_g, ln1_b, ln2_g, ln2_b):
    x = np.asarray(x, np.float32)
    latents = np.asarray(latents, np.float32)
    w_q = np.asarray(w_q, np.float32)
    w_kv = np.asarray(w_kv, np.float32)
    w_out = np.asarray(w_out, np.float32)
    ln1_g = np.asarray(ln1_g, np.float32)
    ln1_b = np.asarray(ln1_b, np.float32)
    ln2_g = np.asarray(ln2_g, np.float32)
    ln2_b = np.asarray(ln2_b, np.float32)

    scale = np.float32(1.0 / np.sqrt(np.sqrt(DH)))
    in_maps = []
    for core in range(8):
        b, g = divmod(core, G)
        cols = slice(g * IC, (g + 1) * IC)
        wq_s = w_q[:, cols] * scale
        wk_s = w_kv[:, :INNER][:, cols] * scale
        wv_s = w_kv[:, INNER:][:, cols]
        in_maps.append({
            "x_b": x[b],
            "lat_b": latents[b],
            "w_q": (ln2_g[:, None] * wq_s).astype(bf16),
            "w_k": (ln1_g[:, None] * wk_s).astype(bf16),
            "w_v": (ln1_g[:, None] * wv_s).astype(bf16),
            "w_o": w_out[cols, :].astype(bf16),
            "b_q": (ln2_b @ wq_s).astype(np.float32),
            "b_k": (ln1_b @ wk_s).astype(np.float32),
            "b_v": (ln1_b @ wv_s).astype(np.float32),
        })
    return in_maps


_NC_CACHE = None


def kernel(x, latents, w_q, w_kv, w_out, ln1_g, ln1_b, ln2_g, ln2_b,
           trace=False):
    global _NC_CACHE
    if _NC_CACHE is None:
        _NC_CACHE = build_nc()
    nc = _NC_CACHE
    in_maps = make_in_maps(x, latents, w_q, w_kv, w_out,
                           ln1_g, ln1_b, ln2_g, ln2_b)
    res = run_bass_kernel_spmd(nc, in_maps, core_ids=list(range(8)),
                               trace=trace)
    out = np.zeros((B, N2, D), np.float32)
    for core in range(8):
        out[core // G] += np.asarray(res.results[core]["out_b"], np.float32)
    if trace:
        return out, res
    return out


# revision 22
# speedup vs baseline: 1.1237x; 1.1237x over previous
"""PerceiverAttentionCA on 8 NeuronCores (Trainium2, Bass/Tile).

Sharding: core i handles batch b = i // 4 and head-group g = i % 4
(4 of the 16 heads = 512 of the 2048 inner columns).  Each core
computes LN1(x[b]) / LN2(latents[b]) in fp32, projects q/k/v for its
heads with bf16 matmuls (LayerNorm gamma and the attention scale are
folded into the weights on the host, beta folded into per-projection
bias vectors), runs softmax attention, and produces the partial output
O_g @ w_out[g-rows].  The host sums the 4 partials per batch.

Structure (v2): LN and projections are interleaved per n2/n1 quarter so
the tensor engine is fed from ~30us onward (HAM stays ramped);
activations cross the PE K-major via HWDGE DMA-transpose (no PE
transpose cycles); softmax skips max-subtraction (scores are O(10)
here; exp stays far inside fp32 range) and gets its denominator from
the activation accumulator; attention and the final projection are
interleaved per 512-row block to keep PE dense through the tail.
"""

import numpy as np
import ml_dtypes

from contextlib import ExitStack

import concourse.bass as bass
import concourse.tile as tile
from concourse import bacc, mybir
from concourse.bass_utils import run_bass_kernel_spmd
from concourse.masks import make_identity

FP32 = mybir.dt.float32
BF16 = mybir.dt.bfloat16
AF = mybir.ActivationFunctionType
ALU = mybir.AluOpType

P = 128
B = 2
N1 = 2048          # x sequence length
N2 = 2048          # latents sequence length
KV = 2048          # x feature dim
D = 3072           # latents feature dim
HEADS = 16
DH = 128
INNER = HEADS * DH  # 2048
G = 4               # head groups (tensor-parallel degree per batch)
HC = HEADS // G     # heads per core = 4
IC = HC * DH        # inner columns per core = 512
EPS = 1e-5

N1C = N1 // P       # 16 n1 chunks of 128
N2C = N2 // P       # 16 n2 chunks of 128
DC = D // P         # 24 K-chunks for q projection
KVC = KV // P       # 16 K-chunks for kv projection
OC = D // 512       # 6 out-column chunks of 512
NQ = 4              # quarters

bf16 = ml_dtypes.bfloat16


def _emit(nc, tc, io, ctx):
    x_d = io["x_b"]
    lat_d = io["lat_b"]
    wq_d = io["w_q"]
    wk_d = io["w_k"]
    wv_d = io["w_v"]
    wo_d = io["w_o"]
    bq_d = io["b_q"]
    bk_d = io["b_k"]
    bv_d = io["b_v"]
    out_d = io["out_b"]

    const = ctx.enter_context(tc.tile_pool(name="const", bufs=1))
    stat = ctx.enter_context(tc.tile_pool(name="stat", bufs=8))

    # persistent activations (bf16, K-major unless noted)
    persist = ctx.enter_context(tc.tile_pool(name="persist", bufs=1))
    qT = persist.tile([P, G, N2], BF16, tag="qT")     # [dh, head, n2]
    kT = persist.tile([P, G, N1], BF16, tag="kT")     # [dh, head, n1]
    vn = persist.tile([P, N1C, IC], BF16, tag="vn")   # natural [n1, inner]

    eps_t = const.tile([P, 1], FP32)
    nc.vector.memset(eps_t, EPS)
    ones_t = const.tile([P, P], BF16)
    nc.vector.memset(ones_t, 1.0)
    idn = const.tile([P, P], BF16)
    make_identity(nc, idn)
    bq_t = const.tile([P, G], FP32)
    bk_t = const.tile([P, G], FP32)
    bv_rep = const.tile([P, IC], FP32)

    def ln_tile(xt, dim):
        """LayerNorm stats for one [128, dim] fp32 tile -> (mu, rstd)."""
        nsub = dim // 512
        st = stat.tile([P, nsub, 6], FP32, tag="st")
        for s in range(nsub):
            nc.vector.bn_stats(out=st[:, s, :], in_=xt[:, s * 512:(s + 1) * 512])
        mv = stat.tile([P, 2], FP32, tag="mv")
        nc.vector.bn_aggr(out=mv, in_=st)
        rstd = stat.tile([P, 1], FP32, tag="rstd")
        nc.scalar.activation(out=rstd, in_=mv[:, 1:2], func=AF.Sqrt, bias=eps_t)
        nc.vector.reciprocal(out=rstd, in_=rstd)
        nmr = stat.tile([P, 1], FP32, tag="nmr")
        nc.vector.tensor_mul(out=nmr, in0=mv[:, 0:1], in1=rstd)
        nc.vector.tensor_scalar_mul(out=nmr, in0=nmr, scalar1=-1.0)
        return rstd, nmr

    def pe_warmup(n_mm=96):
        """Dummy matmuls to fill the LN prologue and ramp the PE clock (HAM)
        before the first real projection; results are discarded."""
        with tc.tile_pool(name="warm", bufs=1) as wpool, \
             tc.tile_pool(name="warm_psum", bufs=1, space="PSUM") as wpsum:
            wa = wpool.tile([P, 512], BF16, tag="warm")
            nc.vector.memset(wa, 0.25)
            wp = wpsum.tile([P, 512], FP32, tag="warmp")
            for i in range(n_mm):
                nc.tensor.matmul(wp, lhsT=wa[:, :P], rhs=wa,
                                 start=(i == 0), stop=(i == n_mm - 1))

    # ---- Phase A/B: quarterized LN + q/k/v projections ----
    with tc.tile_pool(name="lat", bufs=3) as lat_pool, \
         tc.tile_pool(name="xb", bufs=3) as x_pool, \
         tc.tile_pool(name="zt", bufs=3) as z_pool, \
         tc.tile_pool(name="lnT", bufs=1) as lnT_pool, \
         tc.tile_pool(name="xnT", bufs=1) as xnT_pool, \
         tc.tile_pool(name="wq", bufs=2) as wq_pool, \
         tc.tile_pool(name="wk", bufs=2) as wk_pool, \
         tc.tile_pool(name="wv", bufs=1) as wv_pool, \
         tc.tile_pool(name="ab_psum", bufs=4, space="PSUM") as ab_psum, \
         tc.tile_pool(name="t_psum", bufs=3, space="PSUM") as t_psum:

        wq_r = wq_d.rearrange("(c p) m -> p c m", p=P)
        wk_r = wk_d.rearrange("(c p) m -> p c m", p=P)
        wv_sb = wv_pool.tile([P, KVC, IC], BF16, tag="wv")

        pe_warmup(96)

        for q4 in range(NQ):
            # LN2 quarter -> lnTq [P, DC, 512]  (z on DVE, transpose on ACT)
            lnTq = lnT_pool.tile([P, DC, 512], BF16, tag="lnT")
            for ii in range(4):
                i = q4 * 4 + ii
                xt = lat_pool.tile([P, D], FP32, tag="lat")
                nc.sync.dma_start(out=xt, in_=lat_d[i * P:(i + 1) * P, :])
                rstd, nmr = ln_tile(xt, D)
                zt = z_pool.tile([P, D], BF16, tag="zt")
                nc.scalar.activation(out=zt, in_=xt, func=AF.Identity,
                                     bias=nmr, scale=rstd)
                for cg in range(DC // 4):
                    tp = t_psum.tile([P, 4, P], BF16, tag="tp")
                    for cc in range(4):
                        c = cg * 4 + cc
                        nc.tensor.transpose(tp[:, cc, :],
                                            zt[:, c * P:(c + 1) * P], idn)
                    nc.scalar.copy(
                        out=lnTq[:, cg * 4:(cg + 1) * 4, ii * P:(ii + 1) * P],
                        in_=tp)
            # q projection for this n2 quarter
            if q4 == 0:
                nc.sync.dma_start(out=bq_t, in_=bq_d.rearrange("(c p) -> p c", p=P))
            for m in range(G):
                wqm = wq_pool.tile([P, DC, P], BF16, tag="wq")
                nc.sync.dma_start(out=wqm, in_=wq_r[:, :, m * P:(m + 1) * P])
                ps = ab_psum.tile([P, 512], FP32, tag="ab")
                for c in range(DC):
                    nc.tensor.matmul(ps, lhsT=wqm[:, c, :], rhs=lnTq[:, c, :],
                                     start=(c == 0), stop=(c == DC - 1))
                nc.scalar.activation(out=qT[:, m, q4 * 512:(q4 + 1) * 512],
                                     in_=ps, func=AF.Identity,
                                     bias=bq_t[:, m:m + 1])
            # LN1 quarter -> xnTq [P, KVC, 512]  (z on DVE, transpose on ACT)
            xnTq = xnT_pool.tile([P, KVC, 512], BF16, tag="xnT")
            for ii in range(4):
                i = q4 * 4 + ii
                xt = x_pool.tile([P, KV], FP32, tag="xb")
                nc.sync.dma_start(out=xt, in_=x_d[i * P:(i + 1) * P, :])
                rstd, nmr = ln_tile(xt, KV)
                zt = z_pool.tile([P, KV], BF16, tag="zt")
                nc.scalar.activation(out=zt, in_=xt, func=AF.Identity,
                                     bias=nmr, scale=rstd)
                for cg in range(KVC // 4):
                    tp = t_psum.tile([P, 4, P], BF16, tag="tp")
                    for cc in range(4):
                        c = cg * 4 + cc
                        nc.tensor.transpose(tp[:, cc, :],
                                            zt[:, c * P:(c + 1) * P], idn)
                    nc.vector.tensor_copy(
                        out=xnTq[:, cg * 4:(cg + 1) * 4, ii * P:(ii + 1) * P],
                        in_=tp)
            # k projection for this n1 quarter
            if q4 == 0:
                nc.sync.dma_start(out=bk_t, in_=bk_d.rearrange("(c p) -> p c", p=P))
            for m in range(G):
                wkm = wk_pool.tile([P, KVC, P], BF16, tag="wk")
                nc.sync.dma_start(out=wkm, in_=wk_r[:, :, m * P:(m + 1) * P])
                ps = ab_psum.tile([P, 512], FP32, tag="ab")
                for c in range(KVC):
                    nc.tensor.matmul(ps, lhsT=wkm[:, c, :], rhs=xnTq[:, c, :],
                                     start=(c == 0), stop=(c == KVC - 1))
                nc.scalar.activation(out=kT[:, m, q4 * 512:(q4 + 1) * 512],
                                     in_=ps, func=AF.Identity,
                                     bias=bk_t[:, m:m + 1])
            # v rows for this n1 quarter (natural layout)
            if q4 == 0:
                nc.sync.dma_start(
                    out=bv_rep,
                    in_=bass.AP(tensor=bv_d.tensor, offset=bv_d.offset,
                                ap=[[0, P]] + list(bv_d.ap)))
                nc.sync.dma_start(out=wv_sb,
                                  in_=wv_d.rearrange("(c p) m -> p c m", p=P))
            for ii in range(4):
                ps = ab_psum.tile([P, IC], FP32, tag="ab")
                for c in range(KVC):
                    nc.tensor.matmul(ps, lhsT=xnTq[:, c, ii * P:(ii + 1) * P],
                                     rhs=wv_sb[:, c, :],
                                     start=(c == 0), stop=(c == KVC - 1))
                nc.vector.tensor_add(out=vn[:, q4 * 4 + ii, :], in0=ps, in1=bv_rep)

    # ---- Phase C/D: attention + output projection, per 512-row n2 block ----
    with tc.tile_pool(name="PT", bufs=4) as PT_pool, \
         tc.tile_pool(name="rep", bufs=4) as rep_pool, \
         tc.tile_pool(name="OTb", bufs=2) as OTb_pool, \
         tc.tile_pool(name="wo", bufs=1) as wo_pool, \
         tc.tile_pool(name="orow", bufs=2) as orow_pool, \
         tc.tile_pool(name="s_psum", bufs=2, space="PSUM") as s_psum, \
         tc.tile_pool(name="den_psum", bufs=1, space="PSUM") as den_psum, \
         tc.tile_pool(name="o_psum", bufs=1, space="PSUM") as o_psum, \
         tc.tile_pool(name="d_psum", bufs=2, space="PSUM") as d_psum:

        wo_sb = wo_pool.tile([P, G, D], BF16, tag="wo")
        nc.sync.dma_start(out=wo_sb, in_=wo_d.rearrange("(h p) o -> p h o", p=P))

        for nb in range(4):
            OTb = OTb_pool.tile([P, G, 512], BF16, tag="OTb")
            PTbs = []
            reps = []
            # S^T/exp for all 4 heads first: exp writes P^T directly (no
            # transpose needed).
            for h in range(G):
                PTb = PT_pool.tile([P, N1C, 512], BF16, tag="PT")
                PTbs.append(PTb)
                for j2 in range(N1C // 2):
                    ps = s_psum.tile([P, 1024], FP32, tag="s")
                    for jj in range(2):
                        j = j2 * 2 + jj
                        nc.tensor.matmul(ps[:, jj * 512:(jj + 1) * 512],
                                         lhsT=kT[:, h, j * P:(j + 1) * P],
                                         rhs=qT[:, h, nb * 512:(nb + 1) * 512],
                                         start=True, stop=True)
                    nc.scalar.activation(out=PTb[:, j2 * 2:j2 * 2 + 2, :],
                                         in_=ps, func=AF.Exp)
            # denominator: ones-matrix lhsT broadcast-reduces P^T into all
            # 128 partitions at the same PE cost; reciprocal is full-width.
            for h in range(G):
                dn = den_psum.tile([P, 512], FP32, tag="den")
                for j in range(N1C):
                    nc.tensor.matmul(dn, lhsT=ones_t, rhs=PTbs[h][:, j, :],
                                     start=(j == 0), stop=(j == N1C - 1))
                rep = rep_pool.tile([P, 512], FP32, tag="rep")
                nc.vector.reciprocal(out=rep, in_=dn)
                reps.append(rep)
            for h in range(G):
                po = o_psum.tile([P, 512], FP32, tag="o")
                for j in range(N1C):
                    nc.tensor.matmul(po, lhsT=vn[:, j, h * DH:(h + 1) * DH],
                                     rhs=PTbs[h][:, j, :],
                                     start=(j == 0), stop=(j == N1C - 1))
                nc.vector.tensor_mul(out=OTb[:, h, :], in0=po, in1=reps[h])
            # output projection for this block's 4 row-chunks
            for ii in range(4):
                orow = orow_pool.tile([P, D], FP32, tag="orow")
                for oc in range(OC):
                    ps = d_psum.tile([P, 512], FP32, tag="d")
                    for h in range(G):
                        nc.tensor.matmul(ps, lhsT=OTb[:, h, ii * P:(ii + 1) * P],
                                         rhs=wo_sb[:, h, oc * 512:(oc + 1) * 512],
                                         start=(h == 0), stop=(h == G - 1))
                    nc.vector.tensor_copy(out=orow[:, oc * 512:(oc + 1) * 512], in_=ps)
                i = nb * 4 + ii
                nc.sync.dma_start(out=out_d[i * P:(i + 1) * P, :], in_=orow)


def build_nc():
    nc = bacc.Bacc("TRN2", target_bir_lowering=False, debug=False,
                   enable_asserts=False)
    io = {
        "x_b": nc.dram_tensor("x_b", (N1, KV), FP32, kind="ExternalInput").ap(),
        "lat_b": nc.dram_tensor("lat_b", (N2, D), FP32, kind="ExternalInput").ap(),
        "w_q": nc.dram_tensor("w_q", (D, IC), BF16, kind="ExternalInput").ap(),
        "w_k": nc.dram_tensor("w_k", (KV, IC), BF16, kind="ExternalInput").ap(),
        "w_v": nc.dram_tensor("w_v", (KV, IC), BF16, kind="ExternalInput").ap(),
        "w_o": nc.dram_tensor("w_o", (IC, D), BF16, kind="ExternalInput").ap(),
        "b_q": nc.dram_tensor("b_q", (IC,), FP32, kind="ExternalInput").ap(),
        "b_k": nc.dram_tensor("b_k", (IC,), FP32, kind="ExternalInput").ap(),
        "b_v": nc.dram_tensor("b_v", (IC,), FP32, kind="ExternalInput").ap(),
        "out_b": nc.dram_tensor("out_b", (N2, D), FP32, kind="ExternalOutput").ap(),
    }
    with tile.TileContext(nc) as tc, ExitStack() as ctx:
        _emit(nc, tc, io, ctx)
    nc.compile()
    return nc


def make_in_maps(x, latents, w_q, w_kv, w_out, ln1_g, ln1_b, ln2_g, ln2_b):
    x = np.asarray(x, np.float32)
    latents = np.asarray(latents, np.float32)
    w_q = np.asarray(w_q, np.float32)
    w_kv = np.asarray(w_kv, np.float32)
    w_out = np.asarray(w_out, np.float32)
    ln1_g = np.asarray(ln1_g, np.float32)
    ln1_b = np.asarray(ln1_b, np.float32)
    ln2_g = np.asarray(ln2_g, np.float32)
    ln2_b = np.asarray(ln2_b, np.float32)

    scale = np.float32(1.0 / np.sqrt(np.sqrt(DH)))
    in_maps = []
    for core in range(8):
        b, g = divmod(core, G)
        cols = slice(g * IC, (g + 1) * IC)
        wq_s = w_q[:, cols] * scale
        wk_s = w_kv[:, :INNER][:, cols] * scale
        wv_s = w_kv[:, INNER:][:, cols]
        in_maps.append({
            "x_b": x[b],
            "lat_b": latents[b],
            "w_q": (ln2_g[:, None] * wq_s).astype(bf16),
            "w_k": (ln1_g[:, None] * wk_s).astype(bf16),
            "w_v": (ln1_g[:, None] * wv_s).astype(bf16),
            "w_o": w_out[cols, :].astype(bf16),
            "b_q": (ln2_b @ wq_s).astype(np.float32),
            "b_k": (ln1_b @ wk_s).astype(np.float32),
            "b_v": (ln1_b @ wv_s).astype(np.float32),
        })
    return in_maps


_NC_CACHE = None


def kernel(x, latents, w_q, w_kv, w_out, ln1_g, ln1_b, ln2_g, ln2_b,
           trace=False):
    global _NC_CACHE
    if _NC_CACHE is None:
        _NC_CACHE = build_nc()
    nc = _NC_CACHE
    in_maps = make_in_maps(x, latents, w_q, w_kv, w_out,
                           ln1_g, ln1_b, ln2_g, ln2_b)
    res = run_bass_kernel_spmd(nc, in_maps, core_ids=list(range(8)),
                               trace=trace)
    out = np.zeros((B, N2, D), np.float32)
    for core in range(8):
        out[core // G] += np.asarray(res.results[core]["out_b"], np.float32)
    if trace:
        return out, res
    return out


# revision 23
# speedup vs baseline: 1.1520x; 1.0252x over previous
"""PerceiverAttentionCA on 8 NeuronCores (Trainium2, Bass/Tile).

Sharding: core i handles batch b = i // 4 and head-group g = i % 4
(4 of the 16 heads = 512 of the 2048 inner columns).  Each core
computes LN1(x[b]) / LN2(latents[b]) in fp32, projects q/k/v for its
heads with bf16 matmuls (LayerNorm gamma and the attention scale are
folded into the weights on the host, beta folded into per-projection
bias vectors), runs softmax attention, and produces the partial output
O_g @ w_out[g-rows].  The host sums the 4 partials per batch.

Structure (v2): LN and projections are interleaved per n2/n1 quarter so
the tensor engine is fed from ~30us onward (HAM stays ramped);
activations cross the PE K-major via HWDGE DMA-transpose (no PE
transpose cycles); softmax skips max-subtraction (scores are O(10)
here; exp stays far inside fp32 range) and gets its denominator from
the activation accumulator; attention and the final projection are
interleaved per 512-row block to keep PE dense through the tail.
"""

import numpy as np
import ml_dtypes

from contextlib import ExitStack

import concourse.bass as bass
import concourse.tile as tile
from concourse import bacc, mybir
from concourse.bass_utils import run_bass_kernel_spmd
from concourse.masks import make_identity

FP32 = mybir.dt.float32
BF16 = mybir.dt.bfloat16
AF = mybir.ActivationFunctionType
ALU = mybir.AluOpType

P = 128
B = 2
N1 = 2048          # x sequence length
N2 = 2048          # latents sequence length
KV = 2048          # x feature dim
D = 3072           # latents feature dim
HEADS = 16
DH = 128
INNER = HEADS * DH  # 2048
G = 4               # head groups (tensor-parallel degree per batch)
HC = HEADS // G     # heads per core = 4
IC = HC * DH        # inner columns per core = 512
EPS = 1e-5

N1C = N1 // P       # 16 n1 chunks of 128
N2C = N2 // P       # 16 n2 chunks of 128
DC = D // P         # 24 K-chunks for q projection
KVC = KV // P       # 16 K-chunks for kv projection
OC = D // 512       # 6 out-column chunks of 512
NQ = 4              # quarters

bf16 = ml_dtypes.bfloat16


def _emit(nc, tc, io, ctx):
    x_d = io["x_b"]
    lat_d = io["lat_b"]
    wq_d = io["w_q"]
    wk_d = io["w_k"]
    wv_d = io["w_v"]
    wo_d = io["w_o"]
    bq_d = io["b_q"]
    bk_d = io["b_k"]
    bv_d = io["b_v"]
    out_d = io["out_b"]

    const = ctx.enter_context(tc.tile_pool(name="const", bufs=1))
    stat = ctx.enter_context(tc.tile_pool(name="stat", bufs=8))

    # persistent activations (bf16, K-major unless noted)
    persist = ctx.enter_context(tc.tile_pool(name="persist", bufs=1))
    qT = persist.tile([P, G, N2], BF16, tag="qT")     # [dh, head, n2]
    kT = persist.tile([P, G, N1], BF16, tag="kT")     # [dh, head, n1]
    vn = persist.tile([P, N1C, IC], BF16, tag="vn")   # natural [n1, inner]

    eps_t = const.tile([P, 1], FP32)
    nc.vector.memset(eps_t, EPS)
    ones_t = const.tile([P, P], BF16)
    nc.vector.memset(ones_t, 1.0)
    idn = const.tile([P, P], BF16)
    make_identity(nc, idn)
    bq_t = const.tile([P, G], FP32)
    bk_t = const.tile([P, G], FP32)
    bv_rep = const.tile([P, IC], FP32)

    def ln_tile(xt, dim):
        """LayerNorm stats for one [128, dim] fp32 tile -> (mu, rstd)."""
        nsub = dim // 512
        st = stat.tile([P, nsub, 6], FP32, tag="st")
        for s in range(nsub):
            nc.vector.bn_stats(out=st[:, s, :], in_=xt[:, s * 512:(s + 1) * 512])
        mv = stat.tile([P, 2], FP32, tag="mv")
        nc.vector.bn_aggr(out=mv, in_=st)
        rstd = stat.tile([P, 1], FP32, tag="rstd")
        nc.scalar.activation(out=rstd, in_=mv[:, 1:2], func=AF.Sqrt, bias=eps_t)
        nc.vector.reciprocal(out=rstd, in_=rstd)
        nmr = stat.tile([P, 1], FP32, tag="nmr")
        nc.vector.tensor_mul(out=nmr, in0=mv[:, 0:1], in1=rstd)
        nc.vector.tensor_scalar_mul(out=nmr, in0=nmr, scalar1=-1.0)
        return rstd, nmr

    def pe_warmup(n_mm=96):
        """Dummy matmuls to fill the LN prologue and ramp the PE clock (HAM)
        before the first real projection; results are discarded."""
        with tc.tile_pool(name="warm", bufs=1) as wpool, \
             tc.tile_pool(name="warm_psum", bufs=1, space="PSUM") as wpsum:
            wa = wpool.tile([P, 512], BF16, tag="warm")
            nc.vector.memset(wa, 0.25)
            wp = wpsum.tile([P, 512], FP32, tag="warmp")
            for i in range(n_mm):
                nc.tensor.matmul(wp, lhsT=wa[:, :P], rhs=wa,
                                 start=(i == 0), stop=(i == n_mm - 1))

    # ---- Phase A/B: quarterized LN + q/k/v projections ----
    with tc.tile_pool(name="lat", bufs=3) as lat_pool, \
         tc.tile_pool(name="xb", bufs=3) as x_pool, \
         tc.tile_pool(name="zt", bufs=3) as z_pool, \
         tc.tile_pool(name="lnT", bufs=1) as lnT_pool, \
         tc.tile_pool(name="xnT", bufs=1) as xnT_pool, \
         tc.tile_pool(name="wq", bufs=2) as wq_pool, \
         tc.tile_pool(name="wk", bufs=2) as wk_pool, \
         tc.tile_pool(name="wv", bufs=1) as wv_pool, \
         tc.tile_pool(name="ab_psum", bufs=4, space="PSUM") as ab_psum, \
         tc.tile_pool(name="t_psum", bufs=3, space="PSUM") as t_psum:

        wq_r = wq_d.rearrange("(c p) m -> p c m", p=P)
        wk_r = wk_d.rearrange("(c p) m -> p c m", p=P)
        wv_sb = wv_pool.tile([P, KVC, IC], BF16, tag="wv")

        pe_warmup(96)

        for q4 in range(NQ):
            # LN2 quarter -> lnTq [P, DC, 512]  (z on DVE, transpose on ACT)
            lnTq = lnT_pool.tile([P, DC, 512], BF16, tag="lnT")
            for ii in range(4):
                i = q4 * 4 + ii
                xt = lat_pool.tile([P, D], FP32, tag="lat")
                nc.sync.dma_start(out=xt, in_=lat_d[i * P:(i + 1) * P, :])
                rstd, nmr = ln_tile(xt, D)
                zt = z_pool.tile([P, D], BF16, tag="zt")
                nc.scalar.activation(out=zt, in_=xt, func=AF.Identity,
                                     bias=nmr, scale=rstd)
                for cg in range(DC // 4):
                    tp = t_psum.tile([P, 4, P], BF16, tag="tp")
                    for cc in range(4):
                        c = cg * 4 + cc
                        nc.tensor.transpose(tp[:, cc, :],
                                            zt[:, c * P:(c + 1) * P], idn)
                    nc.scalar.copy(
                        out=lnTq[:, cg * 4:(cg + 1) * 4, ii * P:(ii + 1) * P],
                        in_=tp)
            # q projection for this n2 quarter
            if q4 == 0:
                nc.sync.dma_start(out=bq_t, in_=bq_d.rearrange("(c p) -> p c", p=P))
            for m in range(G):
                wqm = wq_pool.tile([P, DC, P], BF16, tag="wq")
                nc.sync.dma_start(out=wqm, in_=wq_r[:, :, m * P:(m + 1) * P])
                ps = ab_psum.tile([P, 512], FP32, tag="ab")
                for c in range(DC):
                    nc.tensor.matmul(ps, lhsT=wqm[:, c, :], rhs=lnTq[:, c, :],
                                     start=(c == 0), stop=(c == DC - 1))
                nc.scalar.activation(out=qT[:, m, q4 * 512:(q4 + 1) * 512],
                                     in_=ps, func=AF.Identity,
                                     bias=bq_t[:, m:m + 1])
            # LN1 quarter -> xnTq [P, KVC, 512]  (z on DVE, transpose on ACT)
            xnTq = xnT_pool.tile([P, KVC, 512], BF16, tag="xnT")
            for ii in range(4):
                i = q4 * 4 + ii
                xt = x_pool.tile([P, KV], FP32, tag="xb")
                nc.sync.dma_start(out=xt, in_=x_d[i * P:(i + 1) * P, :])
                rstd, nmr = ln_tile(xt, KV)
                zt = z_pool.tile([P, KV], BF16, tag="zt")
                nc.scalar.activation(out=zt, in_=xt, func=AF.Identity,
                                     bias=nmr, scale=rstd)
                for cg in range(KVC // 4):
                    tp = t_psum.tile([P, 4, P], BF16, tag="tp")
                    for cc in range(4):
                        c = cg * 4 + cc
                        nc.tensor.transpose(tp[:, cc, :],
                                            zt[:, c * P:(c + 1) * P], idn)
                    nc.vector.tensor_copy(
                        out=xnTq[:, cg * 4:(cg + 1) * 4, ii * P:(ii + 1) * P],
                        in_=tp)
            # k projection for this n1 quarter
            if q4 == 0:
                nc.sync.dma_start(out=bk_t, in_=bk_d.rearrange("(c p) -> p c", p=P))
            for m in range(G):
                wkm = wk_pool.tile([P, KVC, P], BF16, tag="wk")
                nc.sync.dma_start(out=wkm, in_=wk_r[:, :, m * P:(m + 1) * P])
                ps = ab_psum.tile([P, 512], FP32, tag="ab")
                for c in range(KVC):
                    nc.tensor.matmul(ps, lhsT=wkm[:, c, :], rhs=xnTq[:, c, :],
                                     start=(c == 0), stop=(c == KVC - 1))
                nc.scalar.activation(out=kT[:, m, q4 * 512:(q4 + 1) * 512],
                                     in_=ps, func=AF.Identity,
                                     bias=bk_t[:, m:m + 1])
            # v rows for this n1 quarter (natural layout)
            if q4 == 0:
                nc.sync.dma_start(
                    out=bv_rep,
                    in_=bass.AP(tensor=bv_d.tensor, offset=bv_d.offset,
                                ap=[[0, P]] + list(bv_d.ap)))
                nc.sync.dma_start(out=wv_sb,
                                  in_=wv_d.rearrange("(c p) m -> p c m", p=P))
            for ii in range(4):
                ps = ab_psum.tile([P, IC], FP32, tag="ab")
                for c in range(KVC):
                    nc.tensor.matmul(ps, lhsT=xnTq[:, c, ii * P:(ii + 1) * P],
                                     rhs=wv_sb[:, c, :],
                                     start=(c == 0), stop=(c == KVC - 1))
                nc.vector.tensor_add(out=vn[:, q4 * 4 + ii, :], in0=ps, in1=bv_rep)

    # ---- Phase C/D: attention + output projection, per 512-row n2 block ----
    with tc.tile_pool(name="PT", bufs=4) as PT_pool, \
         tc.tile_pool(name="rep", bufs=4) as rep_pool, \
         tc.tile_pool(name="OTb", bufs=2) as OTb_pool, \
         tc.tile_pool(name="wo", bufs=1) as wo_pool, \
         tc.tile_pool(name="orow", bufs=2) as orow_pool, \
         tc.tile_pool(name="s_psum", bufs=3, space="PSUM") as s_psum, \
         tc.tile_pool(name="den_psum", bufs=1, space="PSUM") as den_psum, \
         tc.tile_pool(name="o_psum", bufs=2, space="PSUM") as o_psum, \
         tc.tile_pool(name="d_psum", bufs=2, space="PSUM") as d_psum:

        wo_sb = wo_pool.tile([P, G, D], BF16, tag="wo")
        nc.sync.dma_start(out=wo_sb, in_=wo_d.rearrange("(h p) o -> p h o", p=P))

        for nb in range(4):
            OTb = OTb_pool.tile([P, G, 512], BF16, tag="OTb")
            PTbs = []
            reps = []
            # S^T/exp for all 4 heads first: exp writes P^T directly (no
            # transpose needed).
            for h in range(G):
                PTb = PT_pool.tile([P, N1C, 512], BF16, tag="PT")
                PTbs.append(PTb)
                for j in range(N1C):
                    ps = s_psum.tile([P, 512], FP32, tag="s")
                    nc.tensor.matmul(ps, lhsT=kT[:, h, j * P:(j + 1) * P],
                                     rhs=qT[:, h, nb * 512:(nb + 1) * 512],
                                     start=True, stop=True)
                    nc.scalar.activation(out=PTb[:, j, :], in_=ps, func=AF.Exp)
            # denominator: ones-matrix lhsT broadcast-reduces P^T into all
            # 128 partitions at the same PE cost; reciprocal is full-width.
            for h in range(G):
                dn = den_psum.tile([P, 512], FP32, tag="den")
                for j in range(N1C):
                    nc.tensor.matmul(dn, lhsT=ones_t, rhs=PTbs[h][:, j, :],
                                     start=(j == 0), stop=(j == N1C - 1))
                rep = rep_pool.tile([P, 512], FP32, tag="rep")
                nc.vector.reciprocal(out=rep, in_=dn)
                reps.append(rep)
            for h in range(G):
                po = o_psum.tile([P, 512], FP32, tag="o")
                for j in range(N1C):
                    nc.tensor.matmul(po, lhsT=vn[:, j, h * DH:(h + 1) * DH],
                                     rhs=PTbs[h][:, j, :],
                                     start=(j == 0), stop=(j == N1C - 1))
                nc.vector.tensor_mul(out=OTb[:, h, :], in0=po, in1=reps[h])
            # output projection for this block's 4 row-chunks
            for ii in range(4):
                orow = orow_pool.tile([P, D], FP32, tag="orow")
                for oc in range(OC):
                    ps = d_psum.tile([P, 512], FP32, tag="d")
                    for h in range(G):
                        nc.tensor.matmul(ps, lhsT=OTb[:, h, ii * P:(ii + 1) * P],
                                         rhs=wo_sb[:, h, oc * 512:(oc + 1) * 512],
                                         start=(h == 0), stop=(h == G - 1))
                    if oc % 2 == 0:
                        nc.vector.tensor_copy(out=orow[:, oc * 512:(oc + 1) * 512], in_=ps)
                    else:
                        nc.scalar.copy(out=orow[:, oc * 512:(oc + 1) * 512], in_=ps)
                i = nb * 4 + ii
                nc.sync.dma_start(out=out_d[i * P:(i + 1) * P, :], in_=orow)


def build_nc():
    nc = bacc.Bacc("TRN2", target_bir_lowering=False, debug=False,
                   enable_asserts=False)
    io = {
        "x_b": nc.dram_tensor("x_b", (N1, KV), FP32, kind="ExternalInput").ap(),
        "lat_b": nc.dram_tensor("lat_b", (N2, D), FP32, kind="ExternalInput").ap(),
        "w_q": nc.dram_tensor("w_q", (D, IC), BF16, kind="ExternalInput").ap(),
        "w_k": nc.dram_tensor("w_k", (KV, IC), BF16, kind="ExternalInput").ap(),
        "w_v": nc.dram_tensor("w_v", (KV, IC), BF16, kind="ExternalInput").ap(),
        "w_o": nc.dram_tensor("w_o", (IC, D), BF16, kind="ExternalInput").ap(),
        "b_q": nc.dram_tensor("b_q", (IC,), FP32, kind="ExternalInput").ap(),
        "b_k": nc.dram_tensor("b_k", (IC,), FP32, kind="ExternalInput").ap(),
        "b_v": nc.dram_tensor("b_v", (IC,), FP32, kind="ExternalInput").ap(),
        "out_b": nc.dram_tensor("out_b", (N2, D), FP32, kind="ExternalOutput").ap(),
    }
    with tile.TileContext(nc) as tc, ExitStack() as ctx:
        _emit(nc, tc, io, ctx)
    nc.compile()
    return nc


def make_in_maps(x, latents, w_q, w_kv, w_out, ln1_g, ln1_b, ln2_g, ln2_b):
    x = np.asarray(x, np.float32)
    latents = np.asarray(latents, np.float32)
    w_q = np.asarray(w_q, np.float32)
    w_kv = np.asarray(w_kv, np.float32)
    w_out = np.asarray(w_out, np.float32)
    ln1_g = np.asarray(ln1_g, np.float32)
    ln1_b = np.asarray(ln1_b, np.float32)
    ln2_g = np.asarray(ln2_g, np.float32)
    ln2_b = np.asarray(ln2_b, np.float32)

    scale = np.float32(1.0 / np.sqrt(np.sqrt(DH)))
    in_maps = []
    for core in range(8):
        b, g = divmod(core, G)
        cols = slice(g * IC, (g + 1) * IC)
        wq_s = w_q[:, cols] * scale
        wk_s = w_kv[:, :INNER][:, cols] * scale
        wv_s = w_kv[:, INNER:][:, cols]
        in_maps.append({
            "x_b": x[b],
            "lat_b": latents[b],
            "w_q": (ln2_g[:, None] * wq_s).astype(bf16),
            "w_k": (ln1_g[:, None] * wk_s).astype(bf16),
            "w_v": (ln1_g[:, None] * wv_s).astype(bf16),
            "w_o": w_out[cols, :].astype(bf16),
            "b_q": (ln2_b @ wq_s).astype(np.float32),
            "b_k": (ln1_b @ wk_s).astype(np.float32),
            "b_v": (ln1_b @ wv_s).astype(np.float32),
        })
    return in_maps


_NC_CACHE = None


def kernel(x, latents, w_q, w_kv, w_out, ln1_g, ln1_b, ln2_g, ln2_b,
           trace=False):
    global _NC_CACHE
    if _NC_CACHE is None:
        _NC_CACHE = build_nc()
    nc = _NC_CACHE
    in_maps = make_in_maps(x, latents, w_q, w_kv, w_out,
                           ln1_g, ln1_b, ln2_g, ln2_b)
    res = run_bass_kernel_spmd(nc, in_maps, core_ids=list(range(8)),
                               trace=trace)
    out = np.zeros((B, N2, D), np.float32)
    for core in range(8):
        out[core // G] += np.asarray(res.results[core]["out_b"], np.float32)
    if trace:
        return out, res
    return out


# revision 24
# speedup vs baseline: 1.1645x; 1.0109x over previous
"""PerceiverAttentionCA on 8 NeuronCores (Trainium2, Bass/Tile).

Sharding: core i handles batch b = i // 4 and head-group g = i % 4
(4 of the 16 heads = 512 of the 2048 inner columns).  Each core
computes LN1(x[b]) / LN2(latents[b]) in fp32, projects q/k/v for its
heads with bf16 matmuls (LayerNorm gamma and the attention scale are
folded into the weights on the host, beta folded into per-projection
bias vectors), runs softmax attention, and produces the partial output
O_g @ w_out[g-rows].  The host sums the 4 partials per batch.

Structure: LN and the q/k/v projections are interleaved per n2/n1
quarter so the tensor engine streams from ~10us onward (a short dummy
matmul warmup covers the LN prologue and keeps the PE clock ramped);
normalized activations are moved to K-major layout with PE transposes
(128-cycle blocks, batched 4-wide PSUM->SBUF copybacks on ACT/DVE —
measured much faster than the xbar DMA-transpose path, whose transfers
serialize at ~50GB/s per queue); attention computes S transposed so exp
writes P^T directly with no transpose, skips max-subtraction (scores
are O(10) for LN'd inputs; exp stays far inside fp32 range), and gets
its softmax denominator from ones-matrix matmuls that broadcast the
partition-reduction into all 128 partitions; the division rides the
O-tile eviction.  Attention and the final projection are interleaved
per 512-row block so the PE stays dense through the tail.
Measured: ~641us HW exec per core, rel l2 err 0.0054 vs fp32 reference.
"""

import numpy as np
import ml_dtypes

from contextlib import ExitStack

import concourse.bass as bass
import concourse.tile as tile
from concourse import bacc, mybir
from concourse.bass_utils import run_bass_kernel_spmd
from concourse.masks import make_identity

FP32 = mybir.dt.float32
BF16 = mybir.dt.bfloat16
AF = mybir.ActivationFunctionType
ALU = mybir.AluOpType

P = 128
B = 2
N1 = 2048          # x sequence length
N2 = 2048          # latents sequence length
KV = 2048          # x feature dim
D = 3072           # latents feature dim
HEADS = 16
DH = 128
INNER = HEADS * DH  # 2048
G = 4               # head groups (tensor-parallel degree per batch)
HC = HEADS // G     # heads per core = 4
IC = HC * DH        # inner columns per core = 512
EPS = 1e-5

N1C = N1 // P       # 16 n1 chunks of 128
N2C = N2 // P       # 16 n2 chunks of 128
DC = D // P         # 24 K-chunks for q projection
KVC = KV // P       # 16 K-chunks for kv projection
OC = D // 512       # 6 out-column chunks of 512
NQ = 4              # quarters

bf16 = ml_dtypes.bfloat16


def _emit(nc, tc, io, ctx):
    x_d = io["x_b"]
    lat_d = io["lat_b"]
    wq_d = io["w_q"]
    wk_d = io["w_k"]
    wv_d = io["w_v"]
    wo_d = io["w_o"]
    bq_d = io["b_q"]
    bk_d = io["b_k"]
    bv_d = io["b_v"]
    out_d = io["out_b"]

    const = ctx.enter_context(tc.tile_pool(name="const", bufs=1))
    stat = ctx.enter_context(tc.tile_pool(name="stat", bufs=8))

    # persistent activations (bf16, K-major unless noted)
    persist = ctx.enter_context(tc.tile_pool(name="persist", bufs=1))
    qT = persist.tile([P, G, N2], BF16, tag="qT")     # [dh, head, n2]
    kT = persist.tile([P, G, N1], BF16, tag="kT")     # [dh, head, n1]
    vn = persist.tile([P, N1C, IC], BF16, tag="vn")   # natural [n1, inner]

    eps_t = const.tile([P, 1], FP32)
    nc.vector.memset(eps_t, EPS)
    ones_t = const.tile([P, P], BF16)
    nc.vector.memset(ones_t, 1.0)
    idn = const.tile([P, P], BF16)
    make_identity(nc, idn)
    bq_t = const.tile([P, G], FP32)
    bk_t = const.tile([P, G], FP32)
    bv_rep = const.tile([P, IC], FP32)

    def ln_tile(xt, dim):
        """LayerNorm stats for one [128, dim] fp32 tile -> (mu, rstd)."""
        nsub = dim // 512
        st = stat.tile([P, nsub, 6], FP32, tag="st")
        for s in range(nsub):
            nc.vector.bn_stats(out=st[:, s, :], in_=xt[:, s * 512:(s + 1) * 512])
        mv = stat.tile([P, 2], FP32, tag="mv")
        nc.vector.bn_aggr(out=mv, in_=st)
        rstd = stat.tile([P, 1], FP32, tag="rstd")
        nc.scalar.activation(out=rstd, in_=mv[:, 1:2], func=AF.Sqrt, bias=eps_t)
        nc.vector.reciprocal(out=rstd, in_=rstd)
        nmr = stat.tile([P, 1], FP32, tag="nmr")
        nc.vector.tensor_mul(out=nmr, in0=mv[:, 0:1], in1=rstd)
        nc.vector.tensor_scalar_mul(out=nmr, in0=nmr, scalar1=-1.0)
        return rstd, nmr

    def pe_warmup(n_mm=96):
        """Dummy matmuls to fill the LN prologue and ramp the PE clock (HAM)
        before the first real projection; results are discarded."""
        with tc.tile_pool(name="warm", bufs=1) as wpool, \
             tc.tile_pool(name="warm_psum", bufs=1, space="PSUM") as wpsum:
            wa = wpool.tile([P, 512], BF16, tag="warm")
            nc.vector.memset(wa, 0.25)
            wp = wpsum.tile([P, 512], FP32, tag="warmp")
            for i in range(n_mm):
                nc.tensor.matmul(wp, lhsT=wa[:, :P], rhs=wa,
                                 start=(i == 0), stop=(i == n_mm - 1))

    # ---- Phase A/B: quarterized LN + q/k/v projections ----
    with tc.tile_pool(name="lat", bufs=3) as lat_pool, \
         tc.tile_pool(name="xb", bufs=3) as x_pool, \
         tc.tile_pool(name="zt", bufs=3) as z_pool, \
         tc.tile_pool(name="lnT", bufs=1) as lnT_pool, \
         tc.tile_pool(name="xnT", bufs=1) as xnT_pool, \
         tc.tile_pool(name="wq", bufs=2) as wq_pool, \
         tc.tile_pool(name="wk", bufs=2) as wk_pool, \
         tc.tile_pool(name="wv", bufs=1) as wv_pool, \
         tc.tile_pool(name="ab_psum", bufs=4, space="PSUM") as ab_psum, \
         tc.tile_pool(name="t_psum", bufs=3, space="PSUM") as t_psum:

        wq_r = wq_d.rearrange("(c p) m -> p c m", p=P)
        wk_r = wk_d.rearrange("(c p) m -> p c m", p=P)
        wv_sb = wv_pool.tile([P, KVC, IC], BF16, tag="wv")

        pe_warmup(96)

        for q4 in range(NQ):
            # LN2 quarter -> lnTq [P, DC, 512]  (z on DVE, transpose on ACT)
            lnTq = lnT_pool.tile([P, DC, 512], BF16, tag="lnT")
            for ii in range(4):
                i = q4 * 4 + ii
                xt = lat_pool.tile([P, D], FP32, tag="lat")
                nc.sync.dma_start(out=xt, in_=lat_d[i * P:(i + 1) * P, :])
                rstd, nmr = ln_tile(xt, D)
                zt = z_pool.tile([P, D], BF16, tag="zt")
                nc.scalar.activation(out=zt, in_=xt, func=AF.Identity,
                                     bias=nmr, scale=rstd)
                for cg in range(DC // 4):
                    tp = t_psum.tile([P, 4, P], BF16, tag="tp")
                    for cc in range(4):
                        c = cg * 4 + cc
                        nc.tensor.transpose(tp[:, cc, :],
                                            zt[:, c * P:(c + 1) * P], idn)
                    nc.scalar.copy(
                        out=lnTq[:, cg * 4:(cg + 1) * 4, ii * P:(ii + 1) * P],
                        in_=tp)
            # q projection for this n2 quarter
            if q4 == 0:
                nc.sync.dma_start(out=bq_t, in_=bq_d.rearrange("(c p) -> p c", p=P))
            for m in range(G):
                wqm = wq_pool.tile([P, DC, P], BF16, tag="wq")
                nc.sync.dma_start(out=wqm, in_=wq_r[:, :, m * P:(m + 1) * P])
                ps = ab_psum.tile([P, 512], FP32, tag="ab")
                for c in range(DC):
                    nc.tensor.matmul(ps, lhsT=wqm[:, c, :], rhs=lnTq[:, c, :],
                                     start=(c == 0), stop=(c == DC - 1))
                nc.scalar.activation(out=qT[:, m, q4 * 512:(q4 + 1) * 512],
                                     in_=ps, func=AF.Identity,
                                     bias=bq_t[:, m:m + 1])
            # LN1 quarter -> xnTq [P, KVC, 512]  (z on DVE, transpose on ACT)
            xnTq = xnT_pool.tile([P, KVC, 512], BF16, tag="xnT")
            for ii in range(4):
                i = q4 * 4 + ii
                xt = x_pool.tile([P, KV], FP32, tag="xb")
                nc.sync.dma_start(out=xt, in_=x_d[i * P:(i + 1) * P, :])
                rstd, nmr = ln_tile(xt, KV)
                zt = z_pool.tile([P, KV], BF16, tag="zt")
                nc.scalar.activation(out=zt, in_=xt, func=AF.Identity,
                                     bias=nmr, scale=rstd)
                for cg in range(KVC // 4):
                    tp = t_psum.tile([P, 4, P], BF16, tag="tp")
                    for cc in range(4):
                        c = cg * 4 + cc
                        nc.tensor.transpose(tp[:, cc, :],
                                            zt[:, c * P:(c + 1) * P], idn)
                    nc.vector.tensor_copy(
                        out=xnTq[:, cg * 4:(cg + 1) * 4, ii * P:(ii + 1) * P],
                        in_=tp)
            # k projection for this n1 quarter
            if q4 == 0:
                nc.sync.dma_start(out=bk_t, in_=bk_d.rearrange("(c p) -> p c", p=P))
            for m in range(G):
                wkm = wk_pool.tile([P, KVC, P], BF16, tag="wk")
                nc.sync.dma_start(out=wkm, in_=wk_r[:, :, m * P:(m + 1) * P])
                ps = ab_psum.tile([P, 512], FP32, tag="ab")
                for c in range(KVC):
                    nc.tensor.matmul(ps, lhsT=wkm[:, c, :], rhs=xnTq[:, c, :],
                                     start=(c == 0), stop=(c == KVC - 1))
                nc.scalar.activation(out=kT[:, m, q4 * 512:(q4 + 1) * 512],
                                     in_=ps, func=AF.Identity,
                                     bias=bk_t[:, m:m + 1])
            # v rows for this n1 quarter (natural layout)
            if q4 == 0:
                nc.sync.dma_start(
                    out=bv_rep,
                    in_=bass.AP(tensor=bv_d.tensor, offset=bv_d.offset,
                                ap=[[0, P]] + list(bv_d.ap)))
                nc.sync.dma_start(out=wv_sb,
                                  in_=wv_d.rearrange("(c p) m -> p c m", p=P))
            for ii in range(4):
                ps = ab_psum.tile([P, IC], FP32, tag="ab")
                for c in range(KVC):
                    nc.tensor.matmul(ps, lhsT=xnTq[:, c, ii * P:(ii + 1) * P],
                                     rhs=wv_sb[:, c, :],
                                     start=(c == 0), stop=(c == KVC - 1))
                nc.vector.tensor_add(out=vn[:, q4 * 4 + ii, :], in0=ps, in1=bv_rep)

    # ---- Phase C/D: attention + output projection, per 512-row n2 block ----
    with tc.tile_pool(name="PT", bufs=4) as PT_pool, \
         tc.tile_pool(name="rep", bufs=4) as rep_pool, \
         tc.tile_pool(name="OTb", bufs=2) as OTb_pool, \
         tc.tile_pool(name="wo", bufs=1) as wo_pool, \
         tc.tile_pool(name="orow", bufs=2) as orow_pool, \
         tc.tile_pool(name="s_psum", bufs=3, space="PSUM") as s_psum, \
         tc.tile_pool(name="den_psum", bufs=1, space="PSUM") as den_psum, \
         tc.tile_pool(name="o_psum", bufs=2, space="PSUM") as o_psum, \
         tc.tile_pool(name="d_psum", bufs=2, space="PSUM") as d_psum:

        wo_sb = wo_pool.tile([P, G, D], BF16, tag="wo")
        nc.sync.dma_start(out=wo_sb, in_=wo_d.rearrange("(h p) o -> p h o", p=P))

        for nb in range(4):
            OTb = OTb_pool.tile([P, G, 512], BF16, tag="OTb")
            PTbs = []
            reps = []
            # S^T/exp for all 4 heads first: exp writes P^T directly (no
            # transpose needed).
            for h in range(G):
                PTb = PT_pool.tile([P, N1C, 512], BF16, tag="PT")
                PTbs.append(PTb)
                for j in range(N1C):
                    ps = s_psum.tile([P, 512], FP32, tag="s")
                    nc.tensor.matmul(ps, lhsT=kT[:, h, j * P:(j + 1) * P],
                                     rhs=qT[:, h, nb * 512:(nb + 1) * 512],
                                     start=True, stop=True)
                    nc.scalar.activation(out=PTb[:, j, :], in_=ps, func=AF.Exp)
            # denominator: ones-matrix lhsT broadcast-reduces P^T into all
            # 128 partitions at the same PE cost; reciprocal is full-width.
            for h in range(G):
                dn = den_psum.tile([P, 512], FP32, tag="den")
                for j in range(N1C):
                    nc.tensor.matmul(dn, lhsT=ones_t, rhs=PTbs[h][:, j, :],
                                     start=(j == 0), stop=(j == N1C - 1))
                rep = rep_pool.tile([P, 512], FP32, tag="rep")
                nc.vector.reciprocal(out=rep, in_=dn)
                reps.append(rep)
            for h in range(G):
                po = o_psum.tile([P, 512], FP32, tag="o")
                for j in range(N1C):
                    nc.tensor.matmul(po, lhsT=vn[:, j, h * DH:(h + 1) * DH],
                                     rhs=PTbs[h][:, j, :],
                                     start=(j == 0), stop=(j == N1C - 1))
                nc.vector.tensor_mul(out=OTb[:, h, :], in0=po, in1=reps[h])
            # output projection for this block's 4 row-chunks
            for ii in range(4):
                orow = orow_pool.tile([P, D], FP32, tag="orow")
                for oc in range(OC):
                    ps = d_psum.tile([P, 512], FP32, tag="d")
                    for h in range(G):
                        nc.tensor.matmul(ps, lhsT=OTb[:, h, ii * P:(ii + 1) * P],
                                         rhs=wo_sb[:, h, oc * 512:(oc + 1) * 512],
                                         start=(h == 0), stop=(h == G - 1))
                    if oc % 2 == 0:
                        nc.vector.tensor_copy(out=orow[:, oc * 512:(oc + 1) * 512], in_=ps)
                    else:
                        nc.scalar.copy(out=orow[:, oc * 512:(oc + 1) * 512], in_=ps)
                i = nb * 4 + ii
                nc.sync.dma_start(out=out_d[i * P:(i + 1) * P, :], in_=orow)


def build_nc():
    nc = bacc.Bacc("TRN2", target_bir_lowering=False, debug=False,
                   enable_asserts=False)
    io = {
        "x_b": nc.dram_tensor("x_b", (N1, KV), FP32, kind="ExternalInput").ap(),
        "lat_b": nc.dram_tensor("lat_b", (N2, D), FP32, kind="ExternalInput").ap(),
        "w_q": nc.dram_tensor("w_q", (D, IC), BF16, kind="ExternalInput").ap(),
        "w_k": nc.dram_tensor("w_k", (KV, IC), BF16, kind="ExternalInput").ap(),
        "w_v": nc.dram_tensor("w_v", (KV, IC), BF16, kind="ExternalInput").ap(),
        "w_o": nc.dram_tensor("w_o", (IC, D), BF16, kind="ExternalInput").ap(),
        "b_q": nc.dram_tensor("b_q", (IC,), FP32, kind="ExternalInput").ap(),
        "b_k": nc.dram_tensor("b_k", (IC,), FP32, kind="ExternalInput").ap(),
        "b_v": nc.dram_tensor("b_v", (IC,), FP32, kind="ExternalInput").ap(),
        "out_b": nc.dram_tensor("out_b", (N2, D), FP32, kind="ExternalOutput").ap(),
    }
    with tile.TileContext(nc) as tc, ExitStack() as ctx:
        _emit(nc, tc, io, ctx)
    nc.compile()
    return nc


def make_in_maps(x, latents, w_q, w_kv, w_out, ln1_g, ln1_b, ln2_g, ln2_b):
    x = np.asarray(x, np.float32)
    latents = np.asarray(latents, np.float32)
    w_q = np.asarray(w_q, np.float32)
    w_kv = np.asarray(w_kv, np.float32)
    w_out = np.asarray(w_out, np.float32)
    ln1_g = np.asarray(ln1_g, np.float32)
    ln1_b = np.asarray(ln1_b, np.float32)
    ln2_g = np.asarray(ln2_g, np.float32)
    ln2_b = np.asarray(ln2_b, np.float32)

    scale = np.float32(1.0 / np.sqrt(np.sqrt(DH)))
    in_maps = []
    for core in range(8):
        b, g = divmod(core, G)
        cols = slice(g * IC, (g + 1) * IC)
        wq_s = w_q[:, cols] * scale
        wk_s = w_kv[:, :INNER][:, cols] * scale
        wv_s = w_kv[:, INNER:][:, cols]
        in_maps.append({
            "x_b": x[b],
            "lat_b": latents[b],
            "w_q": (ln2_g[:, None] * wq_s).astype(bf16),
            "w_k": (ln1_g[:, None] * wk_s).astype(bf16),
            "w_v": (ln1_g[:, None] * wv_s).astype(bf16),
            "w_o": w_out[cols, :].astype(bf16),
            "b_q": (ln2_b @ wq_s).astype(np.float32),
            "b_k": (ln1_b @ wk_s).astype(np.float32),
            "b_v": (ln1_b @ wv_s).astype(np.float32),
        })
    return in_maps


_NC_CACHE = None


def kernel(x, latents, w_q, w_kv, w_out, ln1_g, ln1_b, ln2_g, ln2_b,
           trace=False):
    global _NC_CACHE
    if _NC_CACHE is None:
        _NC_CACHE = build_nc()
    nc = _NC_CACHE
    in_maps = make_in_maps(x, latents, w_q, w_kv, w_out,
                           ln1_g, ln1_b, ln2_g, ln2_b)
    res = run_bass_kernel_spmd(nc, in_maps, core_ids=list(range(8)),
                               trace=trace)
    out = np.zeros((B, N2, D), np.float32)
    for core in range(8):
        out[core // G] += np.asarray(res.results[core]["out_b"], np.float32)
    if trace:
        return out, res
    return out
